# revision 9
# baseline (speedup 1.0000x reference)
"""Correlation layer (FlowNet-style) on 8 Trainium2 NeuronCores.

Data-parallel over batch (one element per core). Per core, banded-Gram
matmuls with displacement packing:
  - lhsT = x1 rows [24g-8+rho'' : +32) x 4 w-cols -> M = 32*4 = 128
    (host-blocked: the matmul stationary operand must be one contiguous
    free dim per the BIR verifier)
  - rhs  = x2p rows [24g : 24g+24) x 12 padded-w cols -> N = 24*12 = 288
  - psum[m, n] useful iff rho'' = rr - di + 4 and u - ww in [0, 9).

Perf design (validated with CoreSim cost model + HW repeat-slope timing;
the original version was 99% bound on the SP engine issuing 576 tiny
per-rr window DMAs at ~600ns each, 392us simulated / ~508us HW):
  - bf16 inputs + bf16 matmuls: 1 PE cycle/row instead of 4 (f32), half
    the input DMA traffic. PSUM accumulates f32. bf16 band output.
    (rel err ~4e-3 vs the 2e-2 gate)
  - Band-window DMAs merged 2 rr per descriptor set (pure strides only;
    mixed-stride APs break the tile dep tracker), issue alternating
    SP / Pool(SWDGE); PSUM evac copies alternate DVE / ACT (Pool cannot
    read PSUM).
  - Software-pipelined input prefetch (depth 3) + triple-buffered
    staging so group g+1 compute overlaps group g band-window DMAs.
Simulated 82-85us/exec; measured ~68-78us/exec steady-state on HW
(repeat-slope, 96-exec spans), ~7-8x the original kernel (~508-605us by
the same method). Memory-roofline bound: ~23MB HBM traffic/core/exec
(~65us device floor). HW-validated config notes: WDMA="ygs" (ACT in the
window-DMA rotation) looks better in sim but is ~25us/exec WORSE on HW;
deeper prefetch (>3) clogs the shared DMA device FIFO; PSUM groups >2
matmuls/tile reduce PE-evac overlap.
"""

import math
import numpy as np
from contextlib import ExitStack

B, C, H, W = 8, 128, 128, 192
MD = 4
NDISP = 81

R = 24            # x2p rows per group
NG = 6            # row groups
WW = 4            # output w-cols per block
NWB = W // WW     # 48 blocks
UB = WW + 8       # 12 rhs cols per block
NMM = R * UB      # 288 matmul free size
HP = 152          # x1 padded rows: 8 zero + 128 + 16 zero
X1SL = 32 * W     # per-group x1 slab elems/partition
X2C = W + 8       # 200
WIN = 36                   # band window partitions per rr
WINR = 2                   # rr values merged per band-window DMA
NHALF = 1                  # staging chunks per group (windows launch per chunk)
WBH = NWB // NHALF         # w-blocks per staging chunk
RSH = R * WBH * UB         # staging elems/partition per chunk
NQ = R // WINR             # window DMAs per chunk
WQP = WIN + 4 * (WINR - 1)     # partitions per merged window
WQE_H = WINR * WBH * UB        # free elems per merged window per chunk
GRP_OUT = NHALF * NQ * WQP * WQE_H
OUTSZ = NG * GRP_OUT

IN_DTYPE = "bfloat16"     # "bfloat16" | "float32"
MM_DTYPE = "bfloat16"     # "bfloat16" | "float32" | "float32r" (must match IN unless f32->f32r)
BAND_DTYPE = "bfloat16"   # "float32" | "bfloat16"
# x1 layout: host-blocked per-group slabs [(wb, rho'', ww)] — the matmul
# stationary operand must be a single contiguous free dim (BIR verifier:
# "RHS AP can only have one free dimension"), so unblocked x1 is illegal.
X1_MODE = "slab"
EVAC = "vs"               # per-copy engine cycle: v=DVE, s=ACT (Pool can't read PSUM)
WDMA = "yg"               # window-DMA issue engine cycle: y=SP, s=ACT, g=Pool(SWDGE)
PREFETCH = 3              # input-DMA software pipeline depth (<= input pool bufs)
IN_BUFS = 3
ST_BUFS = 3               # staging pool depth
PSGRP = 2                 # matmuls per PSUM tile (PSGRP banks; ring of 8/PSGRP)

_CACHE = {}


def _build(in_dtype_s, mm_dtype_s, band_dtype_s, evac, wdma, x1_mode, nrep=1):
    import concourse.bass as bass
    import concourse.tile as tile
    from concourse import bacc, mybir

    dtmap = {"bfloat16": mybir.dt.bfloat16, "float32": mybir.dt.float32,
             "float32r": mybir.dt.float32r}
    in_dt = dtmap[in_dtype_s]
    band_dt = dtmap[band_dtype_s]
    f32 = mybir.dt.float32

    nc = bacc.Bacc("TRN2", target_bir_lowering=False, debug=False, num_devices=8)
    if nrep > 1:
        # the NEFF cache hashes HLO structure only (not the embedded BIR);
        # an nrep-dependent input shape keeps repeat variants distinct
        nc.dram_tensor("reptag", [1, 8 * nrep], mybir.dt.float32,
                       kind="ExternalInput")
    x1d = nc.dram_tensor("x1p", [128, NG * X1SL], in_dt, kind="ExternalInput")
    x2d = nc.dram_tensor("x2p", [128, NG * R * X2C], in_dt, kind="ExternalInput")
    outd = nc.dram_tensor("band", [OUTSZ], band_dt, kind="ExternalOutput")

    with tile.TileContext(nc) as tc, ExitStack() as ctx:
        x1pool = ctx.enter_context(tc.tile_pool(name="x1", bufs=IN_BUFS))
        x2pool = ctx.enter_context(tc.tile_pool(name="x2", bufs=IN_BUFS))
        pspool = ctx.enter_context(tc.tile_pool(name="ps", bufs=8 // PSGRP, space="PSUM"))
        stpool = ctx.enter_context(tc.tile_pool(name="st", bufs=ST_BUFS))

        ev = 0
        wd = 0
        x1ts, x2ts = {}, {}

        def prefetch(i):
            g = i % NG
            x1t = x1pool.tile([128, X1SL], in_dt, tag="x1")
            x2t = x2pool.tile([128, R * X2C], in_dt, tag="x2")
            if i == 0:
                # startup: land x2 + the first half of x1 before the rest so
                # the first 24 matmuls start ~2us earlier
                h = X1SL // 2
                nc.sync.dma_start(x1t[:, 0:h], x1d.ap()[:, g * X1SL:g * X1SL + h])
                nc.sync.dma_start(x2t[:], x2d.ap()[:, g * R * X2C:(g + 1) * R * X2C])
                nc.sync.dma_start(x1t[:, h:X1SL],
                                  x1d.ap()[:, g * X1SL + h:(g + 1) * X1SL])
            else:
                nc.sync.dma_start(x1t[:], x1d.ap()[:, g * X1SL:(g + 1) * X1SL])
                nc.sync.dma_start(x2t[:], x2d.ap()[:, g * R * X2C:(g + 1) * R * X2C])
            x1ts[i] = x1t
            x2ts[i] = x2t

        for i in range(PREFETCH):
            prefetch(i)
        for i in range(NG * nrep):
            if i + PREFETCH < NG * nrep:
                prefetch(i + PREFETCH)
            g = i % NG
            x1t, x2t = x1ts.pop(i), x2ts.pop(i)
            x2v = x2t[:].rearrange("p (r u) -> p r u", r=R)
            for h in range(NHALF):
                stt = stpool.tile([128, RSH], band_dt, tag=f"st{h}")
                stv = stt[:].rearrange("p (r b u) -> p r b u", r=R, b=WBH)
                for wpl in range(WBH // PSGRP):
                    pst = pspool.tile([128, PSGRP, 512], f32, tag="ps")
                    for k in range(PSGRP):
                        wb = h * WBH + wpl * PSGRP + k
                        lhsT = x1t[:, wb * 128:(wb + 1) * 128]
                        rhs = x2v[:, :, wb * WW:wb * WW + UB]
                        if mm_dtype_s == "float32r":
                            lhsT = lhsT.bitcast(mybir.dt.float32r)
                            rhs = rhs.bitcast(mybir.dt.float32r)
                        nc.tensor.matmul(pst[:, k, 0:NMM], lhsT, rhs,
                                         start=True, stop=True)
                    src = pst[:, :, 0:NMM].rearrange(
                        "p a (r u) -> p r a u", r=R).copy()
                    dst = stv[:, :, wpl * PSGRP:(wpl + 1) * PSGRP, :]
                    e = evac[ev % len(evac)]
                    ev += 1
                    if e == "v":
                        nc.vector.tensor_copy(dst, src)
                    elif e == "s":
                        nc.scalar.copy(dst, src)
                    else:
                        nc.gpsimd.tensor_copy(dst, src)
                # band window DMAs for this chunk: WINR consecutive rr per
                # DMA (pure strides only — mixed-stride APs break the tile
                # dep tracker), issue rotated across engines. Launching per
                # half-chunk halves the evac->window latency barrier.
                rowlen = WBH * UB
                for q in range(NQ):
                    rr0 = q * WINR
                    src = bass.AP(stt[:].tensor,
                                  (4 * rr0) * RSH + rr0 * rowlen,
                                  [[RSH, WQP], [1, WQE_H]])
                    dst = bass.AP(outd.ap().tensor,
                                  ((g * NHALF + h) * NQ + q) * WQP * WQE_H,
                                  [[WQE_H, WQP], [1, WQE_H]])
                    e = wdma[wd % len(wdma)]
                    wd += 1
                    if e == "y":
                        nc.sync.dma_start(dst, src)
                    elif e == "s":
                        nc.scalar.dma_start(dst, src)
                    else:
                        nc.gpsimd.dma_start(dst, src)

    nc.compile()
    return nc


def _get_nc():
    key = (IN_DTYPE, MM_DTYPE, BAND_DTYPE, EVAC, WDMA, X1_MODE)
    if key not in _CACHE:
        _CACHE[key] = _build(*key)
    return _CACHE[key]


def _prep_inputs(x1, x2):
    import ml_dtypes
    np_dt = ml_dtypes.bfloat16 if IN_DTYPE == "bfloat16" else np.float32
    in_maps = []
    for b in range(x1.shape[0]):
        x1p = np.zeros((128, HP, NWB, WW), np_dt)
        x1p.reshape(128, HP, W)[:, 8:8 + H, :] = x1[b]
        win = np.stack([x1p[:, 24 * g:24 * g + 32] for g in range(NG)], axis=1)
        slabs = win.transpose(0, 1, 3, 2, 4).reshape(128, NG * X1SL)
        x2p = np.zeros((128, NG * R, X2C), np_dt)
        x2p[:, 4:4 + H, 4:4 + W] = x2[b]
        in_maps.append({"x1p": np.ascontiguousarray(slabs),
                        "x2p": np.ascontiguousarray(x2p.reshape(128, NG * R * X2C))})
    return in_maps


def _decode(band, out81):
    """band: per-core [OUTSZ] -> out81 [81, H, W] (scaled later)."""
    raw = np.asarray(band, np.float32).reshape(
        NG, NHALF, NQ, WQP // 4, WW, WINR, WBH, UB)
    bv = np.empty((NG, NHALF, NQ, WINR, 9, WW, WBH, UB), np.float32)
    for b in range(WINR):
        bv[:, :, :, b] = raw[:, :, :, b:b + 9, :, b]
    arr = bv.transpose(0, 2, 3, 4, 5, 1, 6, 7).reshape(NG, R, 9, WW, NWB, UB)
    for ww in range(WW):
        sub = arr[:, :, :, ww, :, ww:ww + 9]          # (g, rr, t, wb, dj)
        tmat = sub.transpose(2, 4, 0, 1, 3).reshape(9, 9, NG * R, NWB)
        for t in range(9):
            di_idx = 8 - t                             # di = 4 - t
            r2lo = di_idx
            out81[di_idx * 9:di_idx * 9 + 9, :, ww::WW] = \
                tmat[t, :, r2lo:r2lo + H, :]
    return out81


def kernel(x1, x2):
    from concourse.bass_utils import run_bass_kernel_spmd

    x1 = np.asarray(x1, np.float32)
    x2 = np.asarray(x2, np.float32)
    nc = _get_nc()
    in_maps = _prep_inputs(x1, x2)
    res = run_bass_kernel_spmd(nc, in_maps, core_ids=list(range(8)))

    inv_sqrt_c = np.float32(1.0 / math.sqrt(C))
    out = np.empty((B, NDISP - 1, H, W), np.float32)
    out81 = np.empty((NDISP, H, W), np.float32)
    for b in range(B):
        _decode(res.results[b]["band"], out81)
        out[b] = np.delete(out81, 40, axis=0) * inv_sqrt_c
    return out


# revision 11
# speedup vs baseline: 1.2465x; 1.2465x over previous
"""Correlation layer (FlowNet-style) on 8 Trainium2 NeuronCores.

Data-parallel over batch (one element per core). Per core, banded-Gram
matmuls with displacement packing:
  - lhsT = x1 rows [24g-8+rho'' : +32) x 4 w-cols -> M = 32*4 = 128
    (host-blocked: the matmul stationary operand must be one contiguous
    free dim per the BIR verifier)
  - rhs  = x2p rows [24g : 24g+24) x 12 padded-w cols -> N = 24*12 = 288
  - psum[m, n] useful iff rho'' = rr - di + 4 and u - ww in [0, 9).

Perf design (validated with CoreSim cost model + HW repeat-slope timing;
the original version was 99% bound on the SP engine issuing 576 tiny
per-rr window DMAs at ~600ns each, 392us simulated / ~508us HW):
  - bf16 inputs + bf16 matmuls: 1 PE cycle/row instead of 4 (f32), half
    the input DMA traffic. PSUM accumulates f32. bf16 band output.
    (rel err ~4e-3 vs the 2e-2 gate)
  - Band-window DMAs merged 2 rr per descriptor set (pure strides only;
    mixed-stride APs break the tile dep tracker), issue alternating
    SP / Pool(SWDGE); PSUM evac copies alternate DVE / ACT (Pool cannot
    read PSUM).
  - Software-pipelined input prefetch (depth 3) + triple-buffered
    staging so group g+1 compute overlaps group g band-window DMAs.
Simulated 81.8us/exec; measured 85.5us/exec steady-state on HW
(interleaved repeat-slope, linear to 0.3% across N={1,97,193}), ~6-7x
the original kernel (~508-605us by the same method). Memory-roofline bound: ~23MB HBM traffic/core/exec
(~65us device floor). HW-validated config notes: WDMA="ygs" (ACT in the
window-DMA rotation) looks better in sim but is ~25us/exec WORSE on HW,
while WDMA="ygg" (Pool-heavy) wins in sim AND paired HW A/B;
deeper prefetch (>3) clogs the shared DMA device FIFO; PSUM groups >2
matmuls/tile reduce PE-evac overlap.
"""

import math
import numpy as np
from contextlib import ExitStack

B, C, H, W = 8, 128, 128, 192
MD = 4
NDISP = 81

R = 24            # x2p rows per group
NG = 6            # row groups
WW = 4            # output w-cols per block
NWB = W // WW     # 48 blocks
UB = WW + 8       # 12 rhs cols per block
NMM = R * UB      # 288 matmul free size
HP = 152          # x1 padded rows: 8 zero + 128 + 16 zero
X1SL = 32 * W     # per-group x1 slab elems/partition
X2C = W + 8       # 200
WIN = 36                   # band window partitions per rr
WINR = 2                   # rr values merged per band-window DMA
NHALF = 1                  # staging chunks per group (windows launch per chunk)
WBH = NWB // NHALF         # w-blocks per staging chunk
RSH = R * WBH * UB         # staging elems/partition per chunk
NQ = R // WINR             # window DMAs per chunk
WQP = WIN + 4 * (WINR - 1)     # partitions per merged window
WQE_H = WINR * WBH * UB        # free elems per merged window per chunk
GRP_OUT = NHALF * NQ * WQP * WQE_H
OUTSZ = NG * GRP_OUT

IN_DTYPE = "bfloat16"     # "bfloat16" | "float32"
MM_DTYPE = "bfloat16"     # "bfloat16" | "float32" | "float32r" (must match IN unless f32->f32r)
BAND_DTYPE = "bfloat16"   # "float32" | "bfloat16"
# x1 layout: host-blocked per-group slabs [(wb, rho'', ww)] — the matmul
# stationary operand must be a single contiguous free dim (BIR verifier:
# "RHS AP can only have one free dimension"), so unblocked x1 is illegal.
X1_MODE = "slab"
EVAC = "vs"               # per-copy engine cycle: v=DVE, s=ACT (Pool can't read PSUM)
WDMA = "ygg"              # window-DMA issue engine cycle: y=SP, s=ACT, g=Pool(SWDGE)
PREFETCH = 3              # input-DMA software pipeline depth (<= input pool bufs)
IN_BUFS = 3
ST_BUFS = 3               # staging pool depth
PSGRP = 2                 # matmuls per PSUM tile (PSGRP banks; ring of 8/PSGRP)

_CACHE = {}


def _build(in_dtype_s, mm_dtype_s, band_dtype_s, evac, wdma, x1_mode, nrep=1):
    import concourse.bass as bass
    import concourse.tile as tile
    from concourse import bacc, mybir

    dtmap = {"bfloat16": mybir.dt.bfloat16, "float32": mybir.dt.float32,
             "float32r": mybir.dt.float32r}
    in_dt = dtmap[in_dtype_s]
    band_dt = dtmap[band_dtype_s]
    f32 = mybir.dt.float32

    nc = bacc.Bacc("TRN2", target_bir_lowering=False, debug=False, num_devices=8)
    if nrep > 1:
        # the NEFF cache hashes HLO structure only (not the embedded BIR);
        # an nrep-dependent input shape keeps repeat variants distinct
        nc.dram_tensor("reptag", [1, 8 * nrep], mybir.dt.float32,
                       kind="ExternalInput")
    x1d = nc.dram_tensor("x1p", [128, NG * X1SL], in_dt, kind="ExternalInput")
    x2d = nc.dram_tensor("x2p", [128, NG * R * X2C], in_dt, kind="ExternalInput")
    outd = nc.dram_tensor("band", [OUTSZ], band_dt, kind="ExternalOutput")

    with tile.TileContext(nc) as tc, ExitStack() as ctx:
        x1pool = ctx.enter_context(tc.tile_pool(name="x1", bufs=IN_BUFS))
        x2pool = ctx.enter_context(tc.tile_pool(name="x2", bufs=IN_BUFS))
        pspool = ctx.enter_context(tc.tile_pool(name="ps", bufs=8 // PSGRP, space="PSUM"))
        stpool = ctx.enter_context(tc.tile_pool(name="st", bufs=ST_BUFS))

        ev = 0
        wd = 0
        x1ts, x2ts = {}, {}

        def prefetch(i):
            g = i % NG
            x1t = x1pool.tile([128, X1SL], in_dt, tag="x1")
            x2t = x2pool.tile([128, R * X2C], in_dt, tag="x2")
            if i == 0:
                # startup: land x2 + the first half of x1 before the rest so
                # the first 24 matmuls start ~2us earlier
                h = X1SL // 2
                nc.sync.dma_start(x1t[:, 0:h], x1d.ap()[:, g * X1SL:g * X1SL + h])
                nc.sync.dma_start(x2t[:], x2d.ap()[:, g * R * X2C:(g + 1) * R * X2C])
                nc.sync.dma_start(x1t[:, h:X1SL],
                                  x1d.ap()[:, g * X1SL + h:(g + 1) * X1SL])
            else:
                nc.sync.dma_start(x1t[:], x1d.ap()[:, g * X1SL:(g + 1) * X1SL])
                nc.sync.dma_start(x2t[:], x2d.ap()[:, g * R * X2C:(g + 1) * R * X2C])
            x1ts[i] = x1t
            x2ts[i] = x2t

        for i in range(PREFETCH):
            prefetch(i)
        for i in range(NG * nrep):
            if i + PREFETCH < NG * nrep:
                prefetch(i + PREFETCH)
            g = i % NG
            x1t, x2t = x1ts.pop(i), x2ts.pop(i)
            x2v = x2t[:].rearrange("p (r u) -> p r u", r=R)
            for h in range(NHALF):
                stt = stpool.tile([128, RSH], band_dt, tag=f"st{h}")
                stv = stt[:].rearrange("p (r b u) -> p r b u", r=R, b=WBH)
                for wpl in range(WBH // PSGRP):
                    pst = pspool.tile([128, PSGRP, 512], f32, tag="ps")
                    for k in range(PSGRP):
                        wb = h * WBH + wpl * PSGRP + k
                        lhsT = x1t[:, wb * 128:(wb + 1) * 128]
                        rhs = x2v[:, :, wb * WW:wb * WW + UB]
                        if mm_dtype_s == "float32r":
                            lhsT = lhsT.bitcast(mybir.dt.float32r)
                            rhs = rhs.bitcast(mybir.dt.float32r)
                        nc.tensor.matmul(pst[:, k, 0:NMM], lhsT, rhs,
                                         start=True, stop=True)
                    src = pst[:, :, 0:NMM].rearrange(
                        "p a (r u) -> p r a u", r=R).copy()
                    dst = stv[:, :, wpl * PSGRP:(wpl + 1) * PSGRP, :]
                    e = evac[ev % len(evac)]
                    ev += 1
                    if e == "v":
                        nc.vector.tensor_copy(dst, src)
                    elif e == "s":
                        nc.scalar.copy(dst, src)
                    else:
                        nc.gpsimd.tensor_copy(dst, src)
                # band window DMAs for this chunk: WINR consecutive rr per
                # DMA (pure strides only — mixed-stride APs break the tile
                # dep tracker), issue rotated across engines. Launching per
                # half-chunk halves the evac->window latency barrier.
                rowlen = WBH * UB
                for q in range(NQ):
                    rr0 = q * WINR
                    src = bass.AP(stt[:].tensor,
                                  (4 * rr0) * RSH + rr0 * rowlen,
                                  [[RSH, WQP], [1, WQE_H]])
                    dst = bass.AP(outd.ap().tensor,
                                  ((g * NHALF + h) * NQ + q) * WQP * WQE_H,
                                  [[WQE_H, WQP], [1, WQE_H]])
                    e = wdma[wd % len(wdma)]
                    wd += 1
                    if e == "y":
                        nc.sync.dma_start(dst, src)
                    elif e == "s":
                        nc.scalar.dma_start(dst, src)
                    else:
                        nc.gpsimd.dma_start(dst, src)

    nc.compile()
    return nc


def _get_nc():
    key = (IN_DTYPE, MM_DTYPE, BAND_DTYPE, EVAC, WDMA, X1_MODE)
    if key not in _CACHE:
        _CACHE[key] = _build(*key)
    return _CACHE[key]


def _prep_inputs(x1, x2):
    import ml_dtypes
    np_dt = ml_dtypes.bfloat16 if IN_DTYPE == "bfloat16" else np.float32
    in_maps = []
    for b in range(x1.shape[0]):
        x1p = np.zeros((128, HP, NWB, WW), np_dt)
        x1p.reshape(128, HP, W)[:, 8:8 + H, :] = x1[b]
        win = np.stack([x1p[:, 24 * g:24 * g + 32] for g in range(NG)], axis=1)
        slabs = win.transpose(0, 1, 3, 2, 4).reshape(128, NG * X1SL)
        x2p = np.zeros((128, NG * R, X2C), np_dt)
        x2p[:, 4:4 + H, 4:4 + W] = x2[b]
        in_maps.append({"x1p": np.ascontiguousarray(slabs),
                        "x2p": np.ascontiguousarray(x2p.reshape(128, NG * R * X2C))})
    return in_maps


def _decode(band, out81):
    """band: per-core [OUTSZ] -> out81 [81, H, W] (scaled later)."""
    raw = np.asarray(band, np.float32).reshape(
        NG, NHALF, NQ, WQP // 4, WW, WINR, WBH, UB)
    bv = np.empty((NG, NHALF, NQ, WINR, 9, WW, WBH, UB), np.float32)
    for b in range(WINR):
        bv[:, :, :, b] = raw[:, :, :, b:b + 9, :, b]
    arr = bv.transpose(0, 2, 3, 4, 5, 1, 6, 7).reshape(NG, R, 9, WW, NWB, UB)
    for ww in range(WW):
        sub = arr[:, :, :, ww, :, ww:ww + 9]          # (g, rr, t, wb, dj)
        tmat = sub.transpose(2, 4, 0, 1, 3).reshape(9, 9, NG * R, NWB)
        for t in range(9):
            di_idx = 8 - t                             # di = 4 - t
            r2lo = di_idx
            out81[di_idx * 9:di_idx * 9 + 9, :, ww::WW] = \
                tmat[t, :, r2lo:r2lo + H, :]
    return out81


def kernel(x1, x2):
    from concourse.bass_utils import run_bass_kernel_spmd

    x1 = np.asarray(x1, np.float32)
    x2 = np.asarray(x2, np.float32)
    nc = _get_nc()
    in_maps = _prep_inputs(x1, x2)
    res = run_bass_kernel_spmd(nc, in_maps, core_ids=list(range(8)))

    inv_sqrt_c = np.float32(1.0 / math.sqrt(C))
    out = np.empty((B, NDISP - 1, H, W), np.float32)
    out81 = np.empty((NDISP, H, W), np.float32)
    for b in range(B):
        _decode(res.results[b]["band"], out81)
        out[b] = np.delete(out81, 40, axis=0) * inv_sqrt_c
    return out
